# revision 1
# baseline (speedup 1.0000x reference)
"""Trainium2 Bass kernel for nn_ClearMeshLoss (8-core SPMD).

Strategy (per sharding hint):
  - chamfer + normal-consistency: shard pred rows (matrix A) and gt rows
    (matrix B) across the 8 cores; each core computes its 1250x10000 block of
    the augmented-matmul "c" matrix (c = 2*a.b - |b|^2, so that
    min_j d_ij = |a_i|^2 - max_j c_ij and argmax c = argmin d), reduces
    row-max on DVE, extracts argmax by a GpSimd scalar_tensor_tensor
    (mask*iota, sum-accumulated), and returns per-row partials.
  - sdf/eikonal: data-parallel over the flattened 200000 elements.
  - edge loss: host does the integer-only edge pairing (sort/segment over the
    int32 faces tensor); the float work (face normals, cosines, relu, sum)
    runs on device over pair-packed operands; watertight is integer-only.
  - host combines the tiny per-core partial outputs into the final scalar.
"""
import numpy as np

# ---------------------------------------------------------------- constants
SDF_W, EIK_W, CH_W, NORM_W, EDGE_W, WT_W = 1.0, 0.1, 1.0, 0.5, 0.3, 0.2
TRUNC, SURF_W, DIH_THR = 0.1, 5.0, 0.5
SIGMA = TRUNC / 3.0

N_CORES = 8

# full-size problem config (hardcoded from the problem spec)
FULL_CFG = dict(
    npts=10000,          # points per cloud
    rows_pad=1280,       # per-core padded row count (10 strips of 128)
    cols_pad=10240,      # padded column count (streamed side)
    super_w=2048,        # PSUM supertile width (4 banks)
    tile_w=512,          # matmul free dim
    sdf_n=200000,        # total sdf elements (B*N)
    sdf_shard=25000,     # per-core sdf elements
    sdf_f=196,           # sdf tile free dim ([128,196] = 25088 >= 25000)
    eik_f=196,           # eikonal diffs per partition row
    pair_cap=122880,     # total edge-pair capacity (8*128*120)
    pair_f=120,          # per-core edge pair tile free dim
)

_PROG_CACHE = {}


def build_program(cfg, phases=("cham", "sdf", "eik", "edge")):
    """Build the (single-core SPMD) Bass/Tile program for one config."""
    from contextlib import ExitStack
    import concourse.bacc as bacc
    import concourse.bass as bass
    import concourse.tile as tile
    from concourse import mybir

    f32 = mybir.dt.float32
    AX = mybir.AxisListType
    OP = mybir.AluOpType
    AF = mybir.ActivationFunctionType

    rows_pad = cfg["rows_pad"]
    cols_pad = cfg["cols_pad"]
    super_w = cfg["super_w"]
    tile_w = cfg["tile_w"]
    sdf_f = cfg["sdf_f"]
    eik_f = cfg["eik_f"]
    pair_f = cfg["pair_f"]

    n_strips = rows_pad // 128
    n_super = cols_pad // super_w
    mm_per_super = super_w // tile_w

    nc = bacc.Bacc("TRN2", target_bir_lowering=False)

    # ---- inputs (per-core values supplied by host) ----
    d_a_pred = nc.dram_tensor("a_pred", [4, rows_pad], f32, kind="ExternalInput")
    d_b_gt = nc.dram_tensor("b_gt", [4, cols_pad], f32, kind="ExternalInput")
    d_a_gt = nc.dram_tensor("a_gt", [4, rows_pad], f32, kind="ExternalInput")
    d_b_pred = nc.dram_tensor("b_pred", [4, cols_pad], f32, kind="ExternalInput")
    d_sdf_pred = nc.dram_tensor("sdf_pred", [128, sdf_f], f32, kind="ExternalInput")
    d_sdf_gt = nc.dram_tensor("sdf_gt", [128, sdf_f], f32, kind="ExternalInput")
    d_eik_pred = nc.dram_tensor("eik_pred", [128 * eik_f + 1], f32, kind="ExternalInput")
    d_eik_gt = nc.dram_tensor("eik_gt", [128, eik_f], f32, kind="ExternalInput")
    d_edge = nc.dram_tensor("edge_in", [18, 128, pair_f], f32, kind="ExternalInput")

    # ---- outputs ----
    # cham_out columns: [0..ns) A rowmax, [ns..2ns) A argmax idx, [2ns..3ns) B rowmax
    d_cham = nc.dram_tensor("cham_out", [128, 3 * n_strips], f32, kind="ExternalOutput")
    # part_out cols: 0 sdf_absdiff, 1 sdf_4e_absdiff, 2 eik_num, 3 eik_cnt, 4 edge_relu
    d_part = nc.dram_tensor("part_out", [128, 8], f32, kind="ExternalOutput")

    with tile.TileContext(nc) as tc, ExitStack() as octx:
        singles = octx.enter_context(tc.tile_pool(name="singles", bufs=1))
        cham_o = singles.tile([128, 3 * n_strips], f32)
        part_o = singles.tile([128, 8], f32)
        nc.vector.memset(part_o, 0.0)

        # ================= chamfer / normal consistency =================
        if "cham" not in phases:
            nc.vector.memset(cham_o, 0.0)
        if "cham" in phases:
          with ExitStack() as ctx:
            cpool = ctx.enter_context(tc.tile_pool(name="cpool", bufs=1))
            strips = ctx.enter_context(tc.tile_pool(name="strips", bufs=2))
            lhsp = ctx.enter_context(tc.tile_pool(name="lhsp", bufs=2))
            rhsb = ctx.enter_context(tc.tile_pool(name="rhsb", bufs=3))
            slotp = ctx.enter_context(tc.tile_pool(name="slotp", bufs=4))
            psum = ctx.enter_context(tc.tile_pool(name="psum", bufs=2, space="PSUM"))

            b_gt_t = cpool.tile([4, cols_pad], f32)
            nc.sync.dma_start(out=b_gt_t, in_=d_b_gt[:, :])

            iota_t = cpool.tile([128, cols_pad], f32)
            nc.gpsimd.iota(out=iota_t[:, :], pattern=[[1, cols_pad]], base=0,
                           channel_multiplier=0,
                           allow_small_or_imprecise_dtypes=True)

            for s in range(n_strips):
                # ---------------- matrix A strip (pred rows x gt cols) ---------
                lhsA = lhsp.tile([4, 128], f32, tag="lhs")
                nc.sync.dma_start(out=lhsA, in_=d_a_pred[:, s * 128:(s + 1) * 128])
                stripA = strips.tile([128, cols_pad], f32, tag="strip")
                slotsA = slotp.tile([128, n_super], f32, tag="slots")
                for c in range(n_super):
                    ps = psum.tile([128, super_w], f32, tag="ps")
                    for m in range(mm_per_super):
                        lo = c * super_w + m * tile_w
                        nc.tensor.matmul(ps[:, m * tile_w:(m + 1) * tile_w],
                                         lhsA[:, :], b_gt_t[:, lo:lo + tile_w],
                                         start=True, stop=True)
                    nc.vector.tensor_reduce(out=slotsA[:, c:c + 1], in_=ps[:, :],
                                            axis=AX.X, op=OP.max)
                    nc.scalar.activation(out=stripA[:, c * super_w:(c + 1) * super_w],
                                         in_=ps[:, :], func=AF.Copy)
                rmaxA = cham_o[:, s:s + 1]
                nc.vector.tensor_reduce(out=rmaxA, in_=slotsA[:, :], axis=AX.X,
                                        op=OP.max)
                nc.vector.scalar_tensor_tensor(
                    out=stripA[:, :], in0=stripA[:, :], scalar=rmaxA,
                    in1=iota_t[:, :], op0=OP.is_ge, op1=OP.mult,
                    accum_out=cham_o[:, n_strips + s:n_strips + s + 1])

                # ---------------- matrix B strip (gt rows x pred cols) ---------
                lhsB = lhsp.tile([4, 128], f32, tag="lhs")
                nc.sync.dma_start(out=lhsB, in_=d_a_gt[:, s * 128:(s + 1) * 128])
                slotsB = slotp.tile([128, n_super], f32, tag="slots")
                for c in range(n_super):
                    rhsB = rhsb.tile([4, super_w], f32, tag="rhsB")
                    nc.sync.dma_start(out=rhsB,
                                      in_=d_b_pred[:, c * super_w:(c + 1) * super_w])
                    ps = psum.tile([128, super_w], f32, tag="ps")
                    for m in range(mm_per_super):
                        nc.tensor.matmul(ps[:, m * tile_w:(m + 1) * tile_w],
                                         lhsB[:, :],
                                         rhsB[:, m * tile_w:(m + 1) * tile_w],
                                         start=True, stop=True)
                    nc.vector.tensor_reduce(out=slotsB[:, c:c + 1], in_=ps[:, :],
                                            axis=AX.X, op=OP.max)
                nc.vector.tensor_reduce(out=cham_o[:, 2 * n_strips + s:2 * n_strips + s + 1],
                                        in_=slotsB[:, :], axis=AX.X, op=OP.max)

        # ================= sdf + eikonal =================
        with ExitStack() as ctx:
            spool = ctx.enter_context(tc.tile_pool(name="spool", bufs=1))

            if "sdf" not in phases:
                nc.vector.memset(part_o[:, 0:2], 0.0)
            if "eik" not in phases:
                nc.vector.memset(part_o[:, 2:3], 0.0)
                nc.vector.memset(part_o[:, 3:4], 1.0)
            if "sdf" in phases:
                _emit_sdf(nc, spool, part_o, d_sdf_pred, d_sdf_gt, sdf_f, f32, AX, OP, AF)
            if "eik" in phases:
                _emit_eik(nc, bass, spool, part_o, d_eik_pred, d_eik_gt, eik_f, f32, AX, OP, AF)

        # ================= edge loss (float part) =================
        if "edge" not in phases:
            nc.vector.memset(part_o[:, 4:5], 0.0)
        if "edge" in phases:
          with ExitStack() as ctx:
            epool = ctx.enter_context(tc.tile_pool(name="epool", bufs=1))
            _emit_edge(nc, epool, part_o, d_edge, pair_f, f32, AX, OP, AF)

        nc.sync.dma_start(out=d_cham[:, :], in_=cham_o[:, :])
        nc.sync.dma_start(out=d_part[:, :], in_=part_o[:, :])

    nc.compile()
    return nc


def _emit_sdf(nc, spool, part_o, d_sdf_pred, d_sdf_gt, sdf_f, f32, AX, OP, AF):
        if True:
            pr = spool.tile([128, sdf_f], f32)
            g = spool.tile([128, sdf_f], f32)
            nc.sync.dma_start(out=pr, in_=d_sdf_pred[:, :])
            nc.sync.dma_start(out=g, in_=d_sdf_gt[:, :])

            prc = spool.tile([128, sdf_f], f32)
            gc = spool.tile([128, sdf_f], f32)
            nc.vector.tensor_scalar(out=prc, in0=pr, scalar1=TRUNC, scalar2=-TRUNC,
                                    op0=OP.min, op1=OP.max)
            nc.vector.tensor_scalar(out=gc, in0=g, scalar1=TRUNC, scalar2=-TRUNC,
                                    op0=OP.min, op1=OP.max)
            diff = spool.tile([128, sdf_f], f32)
            nc.vector.tensor_tensor(out=diff, in0=prc, in1=gc, op=OP.subtract)
            absdiff = spool.tile([128, sdf_f], f32)
            nc.scalar.activation(out=absdiff, in_=diff, func=AF.Abs)
            nc.vector.tensor_reduce(out=part_o[:, 0:1], in_=absdiff, axis=AX.X,
                                    op=OP.add)
            absg = spool.tile([128, sdf_f], f32)
            nc.scalar.activation(out=absg, in_=gc, func=AF.Abs)
            e = spool.tile([128, sdf_f], f32)
            nc.scalar.activation(out=e, in_=absg, func=AF.Exp, scale=-1.0 / SIGMA)
            dead = spool.tile([128, sdf_f], f32)
            nc.vector.scalar_tensor_tensor(out=dead, in0=e, scalar=SURF_W - 1.0,
                                           in1=absdiff, op0=OP.mult, op1=OP.mult,
                                           accum_out=part_o[:, 1:2])


def _emit_eik(nc, bass, spool, part_o, d_eik_pred, d_eik_gt, eik_f, f32, AX, OP, AF):
        if True:
            # eikonal: two shifted (non-overlapping within themselves) loads
            ep0 = spool.tile([128, eik_f], f32)
            ep1 = spool.tile([128, eik_f], f32)
            base = d_eik_pred[:]
            src0 = bass.AP(tensor=base.tensor, offset=0,
                           ap=[[eik_f, 128], [1, eik_f]])
            src1 = bass.AP(tensor=base.tensor, offset=1,
                           ap=[[eik_f, 128], [1, eik_f]])
            nc.sync.dma_start(out=ep0[:, :], in_=src0)
            nc.sync.dma_start(out=ep1[:, :], in_=src1)
            eg = spool.tile([128, eik_f], f32)
            nc.sync.dma_start(out=eg, in_=d_eik_gt[:, :])

            dx = spool.tile([128, eik_f], f32)
            nc.vector.tensor_tensor(out=dx, in0=ep1[:, :],
                                    in1=ep0[:, :], op=OP.subtract)
            absdx = spool.tile([128, eik_f], f32)
            nc.scalar.activation(out=absdx, in_=dx, func=AF.Abs)
            t = spool.tile([128, eik_f], f32)
            nc.vector.tensor_scalar(out=t, in0=absdx, scalar1=-1.0, scalar2=None,
                                    op0=OP.add)
            t2 = spool.tile([128, eik_f], f32)
            nc.vector.tensor_tensor(out=t2, in0=t, in1=t, op=OP.mult)
            abseg = spool.tile([128, eik_f], f32)
            nc.scalar.activation(out=abseg, in_=eg, func=AF.Abs)
            mask = spool.tile([128, eik_f], f32)
            nc.vector.tensor_scalar(out=mask, in0=abseg, scalar1=TRUNC, scalar2=None,
                                    op0=OP.is_lt)
            mt2 = spool.tile([128, eik_f], f32)
            nc.vector.tensor_tensor(out=mt2, in0=t2, in1=mask, op=OP.mult)
            nc.vector.tensor_reduce(out=part_o[:, 2:3], in_=mt2, axis=AX.X,
                                    op=OP.add)
            nc.vector.tensor_reduce(out=part_o[:, 3:4], in_=mask, axis=AX.X,
                                    op=OP.add)


def _emit_edge(nc, epool, part_o, d_edge, pair_f, f32, AX, OP, AF):
        if True:
            ev = epool.tile([128, 18, pair_f], f32)
            for p in range(18):
                nc.sync.dma_start(out=ev[:, p, :], in_=d_edge[p, :, :])

            def tt(op, a, b):
                o = epool.tile([128, pair_f], f32, name=f"tmp_{tt.n}")
                tt.n += 1
                nc.vector.tensor_tensor(out=o, in0=a, in1=b, op=op)
                return o
            tt.n = 0

            def cross(e1, e2):
                # e1, e2: lists of 3 APs
                return [tt(OP.subtract, tt(OP.mult, e1[1], e2[2]),
                           tt(OP.mult, e1[2], e2[1])),
                        tt(OP.subtract, tt(OP.mult, e1[2], e2[0]),
                           tt(OP.mult, e1[0], e2[2])),
                        tt(OP.subtract, tt(OP.mult, e1[0], e2[1]),
                           tt(OP.mult, e1[1], e2[0]))]

            def face_normal(base):
                v0 = [ev[:, base + 0, :], ev[:, base + 1, :], ev[:, base + 2, :]]
                v1 = [ev[:, base + 3, :], ev[:, base + 4, :], ev[:, base + 5, :]]
                v2 = [ev[:, base + 6, :], ev[:, base + 7, :], ev[:, base + 8, :]]
                e1 = [tt(OP.subtract, v1[i], v0[i]) for i in range(3)]
                e2 = [tt(OP.subtract, v2[i], v0[i]) for i in range(3)]
                return cross(e1, e2)

            na = face_normal(0)
            nb = face_normal(9)

            def dot3(a, b):
                s = tt(OP.mult, a[0], b[0])
                s = tt(OP.add, s, tt(OP.mult, a[1], b[1]))
                s = tt(OP.add, s, tt(OP.mult, a[2], b[2]))
                return s

            dot = dot3(na, nb)
            na2 = dot3(na, na)
            nb2 = dot3(nb, nb)
            prod2 = tt(OP.mult, na2, nb2)          # (|na| |nb|)^2
            sa = epool.tile([128, pair_f], f32)
            nc.scalar.activation(out=sa, in_=prod2, func=AF.Sqrt)
            sac = epool.tile([128, pair_f], f32)
            nc.vector.tensor_scalar(out=sac, in0=sa, scalar1=1e-24, scalar2=None,
                                    op0=OP.max)
            rs = epool.tile([128, pair_f], f32)
            nc.vector.reciprocal(out=rs, in_=sac)
            cos = tt(OP.mult, dot, rs)
            relu = epool.tile([128, pair_f], f32)
            nbias = epool.tile([128, 1], f32)
            nc.vector.memset(nbias, -DIH_THR)
            nc.scalar.activation(out=relu, in_=cos, func=AF.Relu, bias=nbias[:, 0:1],
                                 accum_out=part_o[:, 4:5])


def get_program(cfg_key="full"):
    if cfg_key not in _PROG_CACHE:
        _PROG_CACHE[cfg_key] = build_program(FULL_CFG)
    return _PROG_CACHE[cfg_key]


# ================================================================== host side
def _host_prep(inputs, cfg):
    """Build the 8 per-core input maps. Only int indexing / packing here."""
    np_f32 = np.float32
    pred_pts = np.ascontiguousarray(inputs["pred_points"][0], dtype=np_f32)  # [N,3]
    gt_pts = np.ascontiguousarray(inputs["gt_points"][0], dtype=np_f32)
    npts = cfg["npts"]
    rows_pad, cols_pad = cfg["rows_pad"], cfg["cols_pad"]
    shard = npts // N_CORES

    FAR = 1e6

    def a_aug(p):  # [4, n]
        return np.concatenate([p.T, np.ones((1, p.shape[0]), np_f32)], 0)

    def b_aug(p):  # [4, n]
        return np.concatenate([2.0 * p.T, -(p * p).sum(-1)[None, :]], 0)

    def pad_pts(p, n):
        out = np.full((n, 3), FAR, np_f32)
        out[:p.shape[0]] = p
        return out

    b_gt_full = np.ascontiguousarray(b_aug(pad_pts(gt_pts, cols_pad)))
    b_pred_full = np.ascontiguousarray(b_aug(pad_pts(pred_pts, cols_pad)))

    # --- sdf / eikonal shards ---
    pred_sdf = inputs["pred_sdf"].reshape(-1).astype(np_f32)   # [200000]
    gt_sdf = inputs["gt_sdf"].reshape(-1).astype(np_f32)
    n_tot = pred_sdf.shape[0]
    sdf_shard, sdf_f, eik_f = cfg["sdf_shard"], cfg["sdf_f"], cfg["eik_f"]
    n_batch = inputs["pred_sdf"].shape[1]  # 100000 (seam stride)

    # --- edge pairing on host (int32 faces only) ---
    verts = np.asarray(inputs["extracted_vertices"], dtype=np_f32)
    faces = np.asarray(inputs["extracted_faces"], dtype=np.int64)
    V = verts.shape[0]
    Fn = faces.shape[0]
    a = faces
    b = np.roll(faces, -1, axis=1)
    lo = np.minimum(a, b)
    hi = np.maximum(a, b)
    key = (lo * V + hi).reshape(-1)
    fid = np.repeat(np.arange(Fn, dtype=np.int64), 3)
    order = np.argsort(key, kind="stable")
    k = key[order]
    f = fid[order]
    same_next = k[:-1] == k[1:]
    prev = np.concatenate([[False], same_next[:-1]])
    nxt = np.concatenate([same_next[1:], [False]])
    is_pair = same_next & ~prev & ~nxt
    pos = np.nonzero(is_pair)[0]
    fa = f[pos]
    fb = f[pos + 1]
    npairs = int(pos.shape[0])
    # watertight (int only)
    is_start = np.concatenate([[True], k[1:] != k[:-1]])
    starts = np.nonzero(is_start)[0]
    run_len = np.diff(np.concatenate([starts, [k.shape[0]]]))
    total_unique = int(starts.shape[0])
    bad = int((run_len != 2).sum())
    wt = (bad / total_unique) if total_unique > 0 else 0.0

    # pack pair vertex coords [18, pair_cap]
    pair_cap = cfg["pair_cap"]
    n_dev_pairs = min(npairs, pair_cap)
    planes = np.zeros((18, pair_cap), np_f32)
    if n_dev_pairs > 0:
        va = verts[faces[fa[:n_dev_pairs]]]     # [n,3(vert),3(xyz)]
        vb = verts[faces[fb[:n_dev_pairs]]]
        planes[0:9, :n_dev_pairs] = va.reshape(n_dev_pairs, 9).T
        planes[9:18, :n_dev_pairs] = vb.reshape(n_dev_pairs, 9).T
    # leftover pairs (beyond device capacity) handled on host
    edge_extra = 0.0
    if npairs > pair_cap:
        va = verts[faces[fa[pair_cap:]]]
        vb = verts[faces[fb[pair_cap:]]]
        na = np.cross(va[:, 1] - va[:, 0], va[:, 2] - va[:, 0])
        nb = np.cross(vb[:, 1] - vb[:, 0], vb[:, 2] - vb[:, 0])
        na /= np.maximum(np.linalg.norm(na, axis=-1, keepdims=True), 1e-12)
        nb /= np.maximum(np.linalg.norm(nb, axis=-1, keepdims=True), 1e-12)
        cos = (na * nb).sum(-1)
        edge_extra = float(np.maximum(cos - DIH_THR, 0.0).sum())

    pair_f = cfg["pair_f"]
    planes8 = planes.reshape(18, N_CORES, 128 * pair_f).transpose(1, 0, 2) \
                    .reshape(N_CORES, 18, 128, pair_f)

    in_maps = []
    for c in range(N_CORES):
        pr_sh = pad_pts(pred_pts[c * shard:(c + 1) * shard], rows_pad)
        gt_sh = pad_pts(gt_pts[c * shard:(c + 1) * shard], rows_pad)

        sp = np.zeros(128 * sdf_f, np_f32)
        sg = np.zeros(128 * sdf_f, np_f32)
        sl = pred_sdf[c * sdf_shard:(c + 1) * sdf_shard]
        sp[:sl.shape[0]] = sl
        sg[:sl.shape[0]] = gt_sdf[c * sdf_shard:(c + 1) * sdf_shard]

        # eikonal: core covers flat diff positions [c*sdf_shard, c*sdf_shard+128*eik_f)
        ep = np.zeros(128 * eik_f + 1, np_f32)
        src = pred_sdf[c * sdf_shard: c * sdf_shard + 128 * eik_f + 1]
        ep[:src.shape[0]] = src
        eg = np.full(128 * eik_f, 1e9, np_f32)
        gsrc = gt_sdf[c * sdf_shard: c * sdf_shard + 128 * eik_f]
        eg[:gsrc.shape[0]] = gsrc
        # invalidate: local >= sdf_shard ; global seam positions j==n_batch-1 mod n_batch
        locs = np.arange(128 * eik_f)
        glob = locs + c * sdf_shard
        bad_m = (locs >= sdf_shard) | ((glob % n_batch) == n_batch - 1) | \
                (glob >= n_tot - 1)
        eg[bad_m] = 1e9

        in_maps.append({
            "a_pred": np.ascontiguousarray(a_aug(pr_sh)),
            "b_gt": b_gt_full,
            "a_gt": np.ascontiguousarray(a_aug(gt_sh)),
            "b_pred": b_pred_full,
            "sdf_pred": sp.reshape(128, sdf_f),
            "sdf_gt": sg.reshape(128, sdf_f),
            "eik_pred": ep,
            "eik_gt": eg.reshape(128, eik_f),
            "edge_in": np.ascontiguousarray(planes8[c]),
        })

    meta = dict(npairs=npairs, wt=wt, edge_extra=edge_extra, shard=shard)
    return in_maps, meta


def _host_post(inputs, cfg, results, meta):
    npts = cfg["npts"]
    shard = meta["shard"]
    rows_pad = cfg["rows_pad"]
    n_strips = rows_pad // 128

    pred_pts = inputs["pred_points"][0].astype(np.float64)
    gt_pts = inputs["gt_points"][0].astype(np.float64)
    p2 = (pred_pts * pred_pts).sum(-1)
    g2 = (gt_pts * gt_pts).sum(-1)

    rowmaxA = np.empty(npts, np.float64)
    idxA = np.empty(npts, np.int64)
    rowmaxB = np.empty(npts, np.float64)
    for c in range(N_CORES):
        cham = results[c]["cham_out"]  # [128, 3*ns]
        # (p, s) -> local row s*128+p
        rmA = cham[:, 0:n_strips].T.reshape(-1)[:shard]
        ixA = cham[:, n_strips:2 * n_strips].T.reshape(-1)[:shard]
        rmB = cham[:, 2 * n_strips:3 * n_strips].T.reshape(-1)[:shard]
        rowmaxA[c * shard:(c + 1) * shard] = rmA
        idxA[c * shard:(c + 1) * shard] = ixA.astype(np.int64)
        rowmaxB[c * shard:(c + 1) * shard] = rmB

    minA = p2 - rowmaxA
    minB = g2 - rowmaxB
    ch = minA.mean() + minB.mean()

    # normal consistency (host gather + cosine over 10000 rows)
    pn = inputs["pred_normals"][0].astype(np.float64)
    gn = inputs["gt_normals"][0].astype(np.float64)
    idxA = np.clip(idxA, 0, npts - 1)
    matched = gn[idxA]
    eps = 1e-8
    num = (pn * matched).sum(-1)
    den = np.maximum(np.linalg.norm(pn, axis=-1), eps) * \
        np.maximum(np.linalg.norm(matched, axis=-1), eps)
    nrm = float(np.mean(1.0 - np.abs(num / den)))

    parts = np.stack([results[c]["part_out"] for c in range(N_CORES)])  # [8,128,8]
    psum = parts.astype(np.float64).sum(axis=(0, 1))                    # [8]
    sdf = (psum[0] + psum[1]) / float(cfg["sdf_n"])
    eik = (psum[2] / psum[3]) if psum[3] > 0 else 0.0

    npairs = meta["npairs"]
    edge = ((psum[4] + meta["edge_extra"]) / npairs) if npairs > 0 else 0.0

    total = (SDF_W * sdf + EIK_W * eik + CH_W * ch + NORM_W * nrm +
             EDGE_W * edge + WT_W * meta["wt"])
    return np.asarray(np.float32(total))


def kernel(**inputs):
    from concourse.bass_utils import run_bass_kernel_spmd
    cfg = FULL_CFG
    nc = get_program()
    in_maps, meta = _host_prep(inputs, cfg)
    res = run_bass_kernel_spmd(nc, in_maps, core_ids=list(range(N_CORES)))
    return _host_post(inputs, cfg, res.results, meta)



# revision 2
# speedup vs baseline: 9.9823x; 9.9823x over previous
"""Trainium2 Bass kernel for nn_ClearMeshLoss (8-core SPMD).

Strategy:
  - chamfer + normal-consistency: both clouds are sorted by x on the host.
    Each core owns 1250 consecutive sorted query rows (10 strips of 128).
    For each strip, only a rank-aligned window of W=1536 sorted target
    columns (with +-1e9 x sentinels at the ends) is scored with the
    augmented matmul  c = 2*a.b - |b|^2  (max_j c <=> min_j dist).  The
    matmul runs in bf16 hi/lo split (K=11 contraction) at full PE rate;
    DVE reduces each PSUM strip to 48 subtile maxes (width 32).  The host
    picks the top-2 subtiles per row, recomputes those 64 candidate
    distances exactly in fp64 (exact min + argmin), then PROVES the
    banded result optimal via the x-gap bound at the window edges; rows
    that fail the proof (outliers, ~4%) fall back to an exact host scan.
    This is exact for any input distribution.
  - sdf/eikonal: data-parallel over the flattened 200000 elements.
  - edge loss: host does the integer-only edge pairing (sort over int32
    faces); the float work (face normals, cosines, relu, sum) runs on
    GpSimd/ScalarE so it overlaps the DVE chamfer reduces; watertight is
    integer-only on host.
  - host combines the tiny per-core partial outputs into the final scalar.
"""
import numpy as np
import ml_dtypes

BF16 = np.dtype(ml_dtypes.bfloat16)

# ---------------------------------------------------------------- constants
SDF_W, EIK_W, CH_W, NORM_W, EDGE_W, WT_W = 1.0, 0.1, 1.0, 0.5, 0.3, 0.2
TRUNC, SURF_W, DIH_THR = 0.1, 5.0, 0.5
SIGMA = TRUNC / 3.0

N_CORES = 8

FULL_CFG = dict(
    npts=10000,          # points per cloud
    shard=1250,          # query rows per core
    n_strips=10,         # strips of 128 rows (1280 >= 1250)
    win=1536,            # moving window width per strip
    sub=32,              # subtile width for the max reduce
    padl=704,            # left sentinel count in the ext target array
    ext_len=704 + 10000 + 736,
    slice_w=128 * 9 + 1536,   # per-core moving slice width (2688)
    sdf_n=200000,        # total sdf elements (B*N)
    sdf_shard=25000,     # per-core sdf elements
    sdf_f=196,           # sdf tile free dim ([128,196] = 25088 >= 25000)
    eik_f=196,           # eikonal diffs per partition row
    pair_cap=122880,     # total edge-pair capacity (8*128*120)
    pair_f=120,          # per-core edge pair tile free dim
)

_PROG_CACHE = {}


def build_program(cfg):
    from contextlib import ExitStack
    import concourse.bacc as bacc
    import concourse.bass as bass
    import concourse.tile as tile
    from concourse import mybir

    f32 = mybir.dt.float32
    bf16 = mybir.dt.bfloat16
    AX = mybir.AxisListType
    OP = mybir.AluOpType
    AF = mybir.ActivationFunctionType

    n_strips = cfg["n_strips"]
    win = cfg["win"]
    sub = cfg["sub"]
    nsub = win // sub
    slice_w = cfg["slice_w"]
    rows_pad = 128 * n_strips
    sdf_f = cfg["sdf_f"]
    eik_f = cfg["eik_f"]
    pair_f = cfg["pair_f"]

    nc = bacc.Bacc("TRN2", target_bir_lowering=False)

    # ---- inputs ----
    d_aA = nc.dram_tensor("a_a", [11, rows_pad], bf16, kind="ExternalInput")
    d_bA = nc.dram_tensor("b_a", [11, slice_w], bf16, kind="ExternalInput")
    d_aB = nc.dram_tensor("a_b", [11, rows_pad], bf16, kind="ExternalInput")
    d_bB = nc.dram_tensor("b_b", [11, slice_w], bf16, kind="ExternalInput")
    d_sdf_pred = nc.dram_tensor("sdf_pred", [128, sdf_f], f32, kind="ExternalInput")
    d_sdf_gt = nc.dram_tensor("sdf_gt", [128, sdf_f], f32, kind="ExternalInput")
    d_eik_pred = nc.dram_tensor("eik_pred", [128 * eik_f + 1], f32, kind="ExternalInput")
    d_eik_gt = nc.dram_tensor("eik_gt", [128, eik_f], f32, kind="ExternalInput")
    d_edge = nc.dram_tensor("edge_in", [18, 128, pair_f], f32, kind="ExternalInput")

    # ---- outputs ----
    d_chamA = nc.dram_tensor("cham_a", [128, n_strips * nsub], f32, kind="ExternalOutput")
    d_chamB = nc.dram_tensor("cham_b", [128, n_strips * nsub], f32, kind="ExternalOutput")
    # part_out cols: 0 sdf_absdiff, 1 sdf_4e_absdiff, 2 eik_num, 3 eik_cnt, 4 edge_relu
    d_part = nc.dram_tensor("part_out", [128, 8], f32, kind="ExternalOutput")

    with tile.TileContext(nc) as tc, ExitStack() as ctx:
        singles = ctx.enter_context(tc.tile_pool(name="singles", bufs=1))
        cpool = ctx.enter_context(tc.tile_pool(name="cpool", bufs=1))
        epool = ctx.enter_context(tc.tile_pool(name="epool", bufs=1))
        spool = ctx.enter_context(tc.tile_pool(name="spool", bufs=1))
        psum = ctx.enter_context(tc.tile_pool(name="psum", bufs=2, space="PSUM"))

        chamA_o = singles.tile([128, n_strips * nsub], f32)
        chamB_o = singles.tile([128, n_strips * nsub], f32)
        part_o = singles.tile([128, 8], f32)
        nc.vector.memset(part_o, 0.0)

        # ---- input DMAs (issue early; big edge tile first) ----
        ev = epool.tile([128, 18, pair_f], f32)
        src = bass.AP(tensor=d_edge[:, :, :].tensor, offset=0,
                      ap=[[pair_f, 128], [128 * pair_f, 18], [1, pair_f]])
        nc.sync.dma_start(out=ev[:, :, :], in_=src)

        aA_t = cpool.tile([11, rows_pad], bf16)
        bA_t = cpool.tile([11, slice_w], bf16)
        aB_t = cpool.tile([11, rows_pad], bf16)
        bB_t = cpool.tile([11, slice_w], bf16)
        nc.sync.dma_start(out=aA_t, in_=d_aA[:, :])
        nc.sync.dma_start(out=bA_t, in_=d_bA[:, :])
        nc.sync.dma_start(out=aB_t, in_=d_aB[:, :])
        nc.sync.dma_start(out=bB_t, in_=d_bB[:, :])

        # ================= chamfer (banded, both directions) =================
        for s in range(n_strips):
            for (a_t, b_t, out_t) in ((aA_t, bA_t, chamA_o), (aB_t, bB_t, chamB_o)):
                ps = psum.tile([128, win], f32, tag="ps")
                for m in range(win // 512):
                    nc.tensor.matmul(ps[:, m * 512:(m + 1) * 512],
                                     a_t[:, s * 128:(s + 1) * 128],
                                     b_t[:, s * 128 + m * 512: s * 128 + (m + 1) * 512],
                                     start=True, stop=True)
                ps_ap = ps[:, :]
                ps3d = bass.AP(tensor=ps_ap.tensor, offset=ps_ap.offset,
                               ap=[ps_ap.ap[0], [sub, nsub], [1, sub]])
                nc.vector.tensor_reduce(out=out_t[:, s * nsub:(s + 1) * nsub],
                                        in_=ps3d, axis=AX.X, op=OP.max)

        # ================= sdf + eikonal =================
        _emit_sdf(nc, spool, part_o, d_sdf_pred, d_sdf_gt, sdf_f, f32, AX, OP, AF)
        _emit_eik(nc, bass, spool, part_o, d_eik_pred, d_eik_gt, eik_f, f32, AX, OP, AF)

        # ================= edge loss (float part, on GpSimd) =================
        _emit_edge(nc, epool, part_o, ev, pair_f, f32, AX, OP, AF)

        nc.sync.dma_start(out=d_chamA[:, :], in_=chamA_o[:, :])
        nc.sync.dma_start(out=d_chamB[:, :], in_=chamB_o[:, :])
        nc.sync.dma_start(out=d_part[:, :], in_=part_o[:, :])

    nc.compile()
    return nc


def _emit_sdf(nc, spool, part_o, d_sdf_pred, d_sdf_gt, sdf_f, f32, AX, OP, AF):
    pr = spool.tile([128, sdf_f], f32)
    g = spool.tile([128, sdf_f], f32)
    nc.sync.dma_start(out=pr, in_=d_sdf_pred[:, :])
    nc.sync.dma_start(out=g, in_=d_sdf_gt[:, :])

    prc = spool.tile([128, sdf_f], f32)
    gc = spool.tile([128, sdf_f], f32)
    nc.vector.tensor_scalar(out=prc, in0=pr, scalar1=TRUNC, scalar2=-TRUNC,
                            op0=OP.min, op1=OP.max)
    nc.vector.tensor_scalar(out=gc, in0=g, scalar1=TRUNC, scalar2=-TRUNC,
                            op0=OP.min, op1=OP.max)
    diff = spool.tile([128, sdf_f], f32)
    nc.vector.tensor_tensor(out=diff, in0=prc, in1=gc, op=OP.subtract)
    absdiff = spool.tile([128, sdf_f], f32)
    nc.scalar.activation(out=absdiff, in_=diff, func=AF.Abs)
    nc.vector.tensor_reduce(out=part_o[:, 0:1], in_=absdiff, axis=AX.X, op=OP.add)
    absg = spool.tile([128, sdf_f], f32)
    nc.scalar.activation(out=absg, in_=gc, func=AF.Abs)
    e = spool.tile([128, sdf_f], f32)
    nc.scalar.activation(out=e, in_=absg, func=AF.Exp, scale=-1.0 / SIGMA)
    dead = spool.tile([128, sdf_f], f32)
    nc.vector.scalar_tensor_tensor(out=dead, in0=e, scalar=SURF_W - 1.0,
                                   in1=absdiff, op0=OP.mult, op1=OP.mult,
                                   accum_out=part_o[:, 1:2])


def _emit_eik(nc, bass, spool, part_o, d_eik_pred, d_eik_gt, eik_f, f32, AX, OP, AF):
    ep0 = spool.tile([128, eik_f], f32)
    ep1 = spool.tile([128, eik_f], f32)
    base = d_eik_pred[:]
    src0 = bass.AP(tensor=base.tensor, offset=0, ap=[[eik_f, 128], [1, eik_f]])
    src1 = bass.AP(tensor=base.tensor, offset=1, ap=[[eik_f, 128], [1, eik_f]])
    nc.sync.dma_start(out=ep0[:, :], in_=src0)
    nc.sync.dma_start(out=ep1[:, :], in_=src1)
    eg = spool.tile([128, eik_f], f32)
    nc.sync.dma_start(out=eg, in_=d_eik_gt[:, :])

    dx = spool.tile([128, eik_f], f32)
    nc.vector.tensor_tensor(out=dx, in0=ep1[:, :], in1=ep0[:, :], op=OP.subtract)
    absdx = spool.tile([128, eik_f], f32)
    nc.scalar.activation(out=absdx, in_=dx, func=AF.Abs)
    t = spool.tile([128, eik_f], f32)
    nc.vector.tensor_scalar(out=t, in0=absdx, scalar1=-1.0, scalar2=None, op0=OP.add)
    t2 = spool.tile([128, eik_f], f32)
    nc.vector.tensor_tensor(out=t2, in0=t, in1=t, op=OP.mult)
    abseg = spool.tile([128, eik_f], f32)
    nc.scalar.activation(out=abseg, in_=eg, func=AF.Abs)
    mask = spool.tile([128, eik_f], f32)
    nc.vector.tensor_scalar(out=mask, in0=abseg, scalar1=TRUNC, scalar2=None,
                            op0=OP.is_lt)
    mt2 = spool.tile([128, eik_f], f32)
    nc.vector.tensor_tensor(out=mt2, in0=t2, in1=mask, op=OP.mult)
    nc.vector.tensor_reduce(out=part_o[:, 2:3], in_=mt2, axis=AX.X, op=OP.add)
    nc.vector.tensor_reduce(out=part_o[:, 3:4], in_=mask, axis=AX.X, op=OP.add)


def _emit_edge(nc, epool, part_o, ev, pair_f, f32, AX, OP, AF):
    def tt(op, a, b):
        o = epool.tile([128, pair_f], f32, name=f"etmp_{tt.n}")
        tt.n += 1
        nc.gpsimd.tensor_tensor(out=o, in0=a, in1=b, op=op)
        return o
    tt.n = 0

    def cross(e1, e2):
        return [tt(OP.subtract, tt(OP.mult, e1[1], e2[2]), tt(OP.mult, e1[2], e2[1])),
                tt(OP.subtract, tt(OP.mult, e1[2], e2[0]), tt(OP.mult, e1[0], e2[2])),
                tt(OP.subtract, tt(OP.mult, e1[0], e2[1]), tt(OP.mult, e1[1], e2[0]))]

    def face_normal(base):
        v0 = [ev[:, base + 0, :], ev[:, base + 1, :], ev[:, base + 2, :]]
        v1 = [ev[:, base + 3, :], ev[:, base + 4, :], ev[:, base + 5, :]]
        v2 = [ev[:, base + 6, :], ev[:, base + 7, :], ev[:, base + 8, :]]
        e1 = [tt(OP.subtract, v1[i], v0[i]) for i in range(3)]
        e2 = [tt(OP.subtract, v2[i], v0[i]) for i in range(3)]
        return cross(e1, e2)

    na = face_normal(0)
    nb = face_normal(9)

    def dot3(a, b):
        s = tt(OP.mult, a[0], b[0])
        s = tt(OP.add, s, tt(OP.mult, a[1], b[1]))
        s = tt(OP.add, s, tt(OP.mult, a[2], b[2]))
        return s

    dot = dot3(na, nb)
    na2 = dot3(na, na)
    nb2 = dot3(nb, nb)
    prod2 = tt(OP.mult, na2, nb2)          # (|na| |nb|)^2
    sa = epool.tile([128, pair_f], f32)
    nc.scalar.activation(out=sa, in_=prod2, func=AF.Sqrt)
    sac = epool.tile([128, pair_f], f32)
    nc.vector.tensor_scalar(out=sac, in0=sa, scalar1=1e-24, scalar2=None, op0=OP.max)
    rs = epool.tile([128, pair_f], f32)
    nc.vector.reciprocal(out=rs, in_=sac)
    cos = epool.tile([128, pair_f], f32)
    nc.vector.tensor_tensor(out=cos, in0=dot, in1=rs, op=OP.mult)
    relu = epool.tile([128, pair_f], f32)
    nbias = epool.tile([128, 1], f32)
    nc.vector.memset(nbias, -DIH_THR)
    nc.scalar.activation(out=relu, in_=cos, func=AF.Relu, bias=nbias[:, 0:1],
                         accum_out=part_o[:, 4:5])


def get_program(cfg_key="full"):
    if cfg_key not in _PROG_CACHE:
        _PROG_CACHE[cfg_key] = build_program(FULL_CFG)
    return _PROG_CACHE[cfg_key]


# ================================================================== host side
def _hi_lo(x):
    h = x.astype(BF16)
    l = (x - h.astype(np.float64)).astype(BF16)
    return h, l


def _build_lhs(a):
    """a: [n,3] fp64 -> [11,n] bf16 rows [ah3, ah3, al3, 1, 1]."""
    ah, al = _hi_lo(a)
    ones = np.ones((1, a.shape[0]), BF16)
    return np.ascontiguousarray(
        np.concatenate([ah.T, ah.T, al.T, ones, ones], 0))


def _build_rhs(b):
    """b: [m,3] fp64 -> [11,m] bf16 rows [2bh3, 2bl3, 2bh3, -sh, -sl]."""
    bh = b.astype(BF16)
    bl2 = (2.0 * (b - bh.astype(np.float64))).astype(BF16)
    bh2 = (2.0 * bh.astype(np.float64)).astype(BF16)
    s = (b * b).sum(-1)
    sh = s.astype(BF16)
    sl = (s - sh.astype(np.float64)).astype(BF16)
    neg_sh = (-sh.astype(np.float64)).astype(BF16)
    neg_sl = (-sl.astype(np.float64)).astype(BF16)
    return np.ascontiguousarray(
        np.concatenate([bh2.T, bl2.T, bh2.T, neg_sh[None], neg_sl[None]], 0))


def _host_prep(inputs, cfg):
    np_f32 = np.float32
    npts = cfg["npts"]
    shard = cfg["shard"]
    n_strips = cfg["n_strips"]
    rows_pad = 128 * n_strips
    slice_w = cfg["slice_w"]
    padl = cfg["padl"]
    ext_len = cfg["ext_len"]

    pred_pts = np.asarray(inputs["pred_points"][0], dtype=np.float64)
    gt_pts = np.asarray(inputs["gt_points"][0], dtype=np.float64)

    pperm = np.argsort(pred_pts[:, 0], kind="stable")
    gperm = np.argsort(gt_pts[:, 0], kind="stable")
    ps = pred_pts[pperm]
    gs = gt_pts[gperm]

    def make_ext(sorted_pts):
        ext = np.empty((ext_len, 3))
        ext[:padl] = [-1e9, 0.0, 0.0]
        ext[padl:padl + npts] = sorted_pts
        ext[padl + npts:] = [1e9, 0.0, 0.0]
        return ext

    g_ext = make_ext(gs)
    p_ext = make_ext(ps)

    def pad_rows(x, n):
        out = np.zeros((n, 3))
        out[:x.shape[0]] = x
        return out

    rhs_gt = _build_rhs(g_ext)     # [11, ext_len]
    rhs_pr = _build_rhs(p_ext)

    # --- sdf / eikonal shards (unsorted originals) ---
    pred_sdf = inputs["pred_sdf"].reshape(-1).astype(np_f32)
    gt_sdf = inputs["gt_sdf"].reshape(-1).astype(np_f32)
    n_tot = pred_sdf.shape[0]
    sdf_shard, sdf_f, eik_f = cfg["sdf_shard"], cfg["sdf_f"], cfg["eik_f"]
    n_batch = inputs["pred_sdf"].shape[1]

    # --- edge pairing on host (int32 faces only) ---
    verts = np.asarray(inputs["extracted_vertices"], dtype=np_f32)
    faces = np.asarray(inputs["extracted_faces"], dtype=np.int64)
    V = verts.shape[0]
    Fn = faces.shape[0]
    a = faces
    b = np.roll(faces, -1, axis=1)
    lo = np.minimum(a, b)
    hi = np.maximum(a, b)
    key = (lo * V + hi).reshape(-1)
    fid = np.repeat(np.arange(Fn, dtype=np.int64), 3)
    order = np.argsort(key, kind="stable")
    k = key[order]
    f = fid[order]
    same_next = k[:-1] == k[1:]
    prev = np.concatenate([[False], same_next[:-1]])
    nxt = np.concatenate([same_next[1:], [False]])
    is_pair = same_next & ~prev & ~nxt
    pos = np.nonzero(is_pair)[0]
    fa = f[pos]
    fb = f[pos + 1]
    npairs = int(pos.shape[0])
    is_start = np.concatenate([[True], k[1:] != k[:-1]])
    starts = np.nonzero(is_start)[0]
    run_len = np.diff(np.concatenate([starts, [k.shape[0]]]))
    total_unique = int(starts.shape[0])
    bad = int((run_len != 2).sum())
    wt = (bad / total_unique) if total_unique > 0 else 0.0

    pair_cap = cfg["pair_cap"]
    n_dev_pairs = min(npairs, pair_cap)
    planes = np.zeros((18, pair_cap), np_f32)
    if n_dev_pairs > 0:
        va = verts[faces[fa[:n_dev_pairs]]]
        vb = verts[faces[fb[:n_dev_pairs]]]
        planes[0:9, :n_dev_pairs] = va.reshape(n_dev_pairs, 9).T
        planes[9:18, :n_dev_pairs] = vb.reshape(n_dev_pairs, 9).T
    edge_extra = 0.0
    if npairs > pair_cap:
        va = verts[faces[fa[pair_cap:]]]
        vb = verts[faces[fb[pair_cap:]]]
        na = np.cross(va[:, 1] - va[:, 0], va[:, 2] - va[:, 0])
        nb = np.cross(vb[:, 1] - vb[:, 0], vb[:, 2] - vb[:, 0])
        na /= np.maximum(np.linalg.norm(na, axis=-1, keepdims=True), 1e-12)
        nb /= np.maximum(np.linalg.norm(nb, axis=-1, keepdims=True), 1e-12)
        cosv = (na * nb).sum(-1)
        edge_extra = float(np.maximum(cosv - DIH_THR, 0.0).sum())

    pair_f = cfg["pair_f"]
    planes8 = planes.reshape(18, N_CORES, 128 * pair_f).transpose(1, 0, 2) \
                    .reshape(N_CORES, 18, 128, pair_f)

    in_maps = []
    for c in range(N_CORES):
        lhsA = _build_lhs(pad_rows(ps[c * shard:(c + 1) * shard], rows_pad))
        lhsB = _build_lhs(pad_rows(gs[c * shard:(c + 1) * shard], rows_pad))
        bA = np.ascontiguousarray(rhs_gt[:, c * shard: c * shard + slice_w])
        bB = np.ascontiguousarray(rhs_pr[:, c * shard: c * shard + slice_w])

        sp = np.zeros(128 * sdf_f, np_f32)
        sg = np.zeros(128 * sdf_f, np_f32)
        sl = pred_sdf[c * sdf_shard:(c + 1) * sdf_shard]
        sp[:sl.shape[0]] = sl
        sg[:sl.shape[0]] = gt_sdf[c * sdf_shard:(c + 1) * sdf_shard]

        ep = np.zeros(128 * eik_f + 1, np_f32)
        src = pred_sdf[c * sdf_shard: c * sdf_shard + 128 * eik_f + 1]
        ep[:src.shape[0]] = src
        eg = np.full(128 * eik_f, 1e9, np_f32)
        gsrc = gt_sdf[c * sdf_shard: c * sdf_shard + 128 * eik_f]
        eg[:gsrc.shape[0]] = gsrc
        locs = np.arange(128 * eik_f)
        glob = locs + c * sdf_shard
        bad_m = (locs >= sdf_shard) | ((glob % n_batch) == n_batch - 1) | \
                (glob >= n_tot - 1)
        eg[bad_m] = 1e9

        in_maps.append({
            "a_a": lhsA,
            "b_a": bA,
            "a_b": lhsB,
            "b_b": bB,
            "sdf_pred": sp.reshape(128, sdf_f),
            "sdf_gt": sg.reshape(128, sdf_f),
            "eik_pred": ep,
            "eik_gt": eg.reshape(128, eik_f),
            "edge_in": np.ascontiguousarray(planes8[c]),
        })

    meta = dict(npairs=npairs, wt=wt, edge_extra=edge_extra,
                pperm=pperm, gperm=gperm, ps=ps, gs=gs,
                p_ext=p_ext, g_ext=g_ext)
    return in_maps, meta


def _refine_side(cfg, results, key, qs, ext, t_sorted, need_idx):
    """Host top-2 subtile refinement + optimality proof + fallback.

    qs: sorted query points [N,3] fp64; ext: target ext array [ext_len,3];
    t_sorted: sorted target points [N,3].  Returns (d2_min[N], argmin_rank[N],
    n_fallback) in SORTED-query order.
    """
    npts = cfg["npts"]
    shard = cfg["shard"]
    n_strips = cfg["n_strips"]
    sub = cfg["sub"]
    nsub = cfg["win"] // sub
    padl = cfg["padl"]
    ext_len = cfg["ext_len"]
    W = cfg["win"]

    M = np.empty((npts, nsub), np.float32)
    wstart = np.empty(npts, np.int64)
    for c in range(N_CORES):
        cham = np.asarray(results[c][key])            # [128, n_strips*nsub]
        # [p, s*nsub+t] -> local row s*128+p
        loc = cham.reshape(128, n_strips, nsub).transpose(1, 0, 2).reshape(-1, nsub)
        M[c * shard:(c + 1) * shard] = loc[:shard]
        lr = np.arange(shard)
        wstart[c * shard:(c + 1) * shard] = c * shard + 128 * (lr // 128)

    top2 = np.argpartition(-M, 1, axis=1)[:, :2]
    cand = wstart[:, None, None] + top2[:, :, None] * sub + np.arange(sub)[None, None, :]
    cand = cand.reshape(npts, 2 * sub)
    tc = ext[cand]
    d2 = ((qs[:, None, :] - tc) ** 2).sum(-1)
    kk = np.argmin(d2, axis=1)
    dmin = d2[np.arange(npts), kk]
    ecol = cand[np.arange(npts), kk]

    tx = ext[:, 0]
    safeL = np.where(wstart == 0, np.inf, qs[:, 0] - tx[np.maximum(wstart - 1, 0)])
    wend = wstart + W
    safeR = np.where(wend >= ext_len, np.inf, tx[np.minimum(wend, ext_len - 1)] - qs[:, 0])
    safe = np.maximum(np.minimum(safeL, safeR), 0.0)
    fb = np.nonzero(dmin > safe * safe)[0]
    if fb.size:
        d2f = ((qs[fb, None, :] - t_sorted[None, :, :]) ** 2).sum(-1)
        kf = np.argmin(d2f, axis=1)
        dmin[fb] = d2f[np.arange(fb.size), kf]
        ecol[fb] = kf + padl
    rank = ecol - padl
    return dmin, rank, int(fb.size)


def _host_post(inputs, cfg, results, meta):
    npts = cfg["npts"]
    pperm, gperm = meta["pperm"], meta["gperm"]
    ps, gs = meta["ps"], meta["gs"]

    dA, rankA, _ = _refine_side(cfg, results, "cham_a", ps, meta["g_ext"], gs, True)
    dB, _, _ = _refine_side(cfg, results, "cham_b", gs, meta["p_ext"], ps, False)
    ch = dA.mean() + dB.mean()

    # normal consistency: map sorted-query rows back to original indices
    idxA = np.empty(npts, np.int64)
    idxA[pperm] = gperm[np.clip(rankA, 0, npts - 1)]
    pn = inputs["pred_normals"][0].astype(np.float64)
    gn = inputs["gt_normals"][0].astype(np.float64)
    matched = gn[idxA]
    eps = 1e-8
    num = (pn * matched).sum(-1)
    den = np.maximum(np.linalg.norm(pn, axis=-1), eps) * \
        np.maximum(np.linalg.norm(matched, axis=-1), eps)
    nrm = float(np.mean(1.0 - np.abs(num / den)))

    parts = np.stack([np.asarray(results[c]["part_out"]) for c in range(N_CORES)])
    psum = parts.astype(np.float64).sum(axis=(0, 1))
    sdf = (psum[0] + psum[1]) / float(cfg["sdf_n"])
    eik = (psum[2] / psum[3]) if psum[3] > 0 else 0.0

    npairs = meta["npairs"]
    edge = ((psum[4] + meta["edge_extra"]) / npairs) if npairs > 0 else 0.0

    total = (SDF_W * sdf + EIK_W * eik + CH_W * ch + NORM_W * nrm +
             EDGE_W * edge + WT_W * meta["wt"])
    return np.asarray(np.float32(total))


def kernel(**inputs):
    from concourse.bass_utils import run_bass_kernel_spmd
    cfg = FULL_CFG
    nc = get_program()
    in_maps, meta = _host_prep(inputs, cfg)
    res = run_bass_kernel_spmd(nc, in_maps, core_ids=list(range(N_CORES)))
    return _host_post(inputs, cfg, res.results, meta)


# revision 4
# speedup vs baseline: 15.1064x; 1.5133x over previous
"""Trainium2 Bass kernel for nn_ClearMeshLoss (8-core SPMD).

Strategy:
  - chamfer + normal-consistency: both clouds are sorted by x on the host.
    Each core owns 1250 consecutive sorted query rows (10 strips of 128).
    For each strip, only a rank-aligned window of W=1024 sorted target
    columns (with +-1e9 x sentinels at the ends) is scored with the
    augmented matmul  c = 2*a.b - |b|^2  (max_j c <=> min_j dist).  The
    matmul runs in bf16 hi/lo split (K=11 contraction) at full PE rate;
    DVE reduces each PSUM strip to 32 subtile maxes (width 32).  The host
    picks the top-2 subtiles per row, recomputes those 64 candidate
    distances exactly in fp64 (exact min + argmin), then PROVES the
    banded result optimal via the x-gap bound at the window edges; rows
    that fail the proof fall back to an exact bounded re-scan on the host
    (|dx| <= sqrt(d_band) window).  Exact for any input distribution.
  - sdf/eikonal: data-parallel over the flattened 200000 elements;
    elementwise stages on GpSimd, reductions on DVE, abs/exp on ScalarE.
  - edge loss: host does the integer-only edge pairing (sort over int32
    faces); the float work (face normals, cosines, relu, sum) runs mostly
    on GpSimd with component-packed [128,3,120] views so it overlaps the
    DVE chamfer reduces; watertight is integer-only on host.
  - host combines the tiny per-core partial outputs into the final scalar.
"""
import numpy as np
import ml_dtypes

BF16 = np.dtype(ml_dtypes.bfloat16)

# ---------------------------------------------------------------- constants
SDF_W, EIK_W, CH_W, NORM_W, EDGE_W, WT_W = 1.0, 0.1, 1.0, 0.5, 0.3, 0.2
TRUNC, SURF_W, DIH_THR = 0.1, 5.0, 0.5
SIGMA = TRUNC / 3.0

N_CORES = 8

FULL_CFG = dict(
    npts=10000,          # points per cloud
    shard=1250,          # query rows per core
    n_strips=10,         # strips of 128 rows (1280 >= 1250)
    win=1024,            # moving window width per strip
    sub=32,              # subtile width for the max reduce
    padl=448,            # left sentinel count in the ext target array
    ext_len=448 + 10000 + 480,
    slice_w=128 * 9 + 1024,   # per-core moving slice width (2176)
    sdf_n=200000,        # total sdf elements (B*N)
    sdf_shard=25000,     # per-core sdf elements
    sdf_f=196,           # sdf tile free dim ([128,196] = 25088 >= 25000)
    eik_f=196,           # eikonal diffs per partition row
    pair_cap=122880,     # total edge-pair capacity (8*128*120)
    pair_f=120,          # per-core edge pair tile free dim
)

_PROG_CACHE = {}


def build_program(cfg):
    from contextlib import ExitStack
    import concourse.bacc as bacc
    import concourse.bass as bass
    import concourse.tile as tile
    from concourse import mybir

    f32 = mybir.dt.float32
    bf16 = mybir.dt.bfloat16
    AX = mybir.AxisListType
    OP = mybir.AluOpType
    AF = mybir.ActivationFunctionType

    n_strips = cfg["n_strips"]
    win = cfg["win"]
    sub = cfg["sub"]
    nsub = win // sub
    slice_w = cfg["slice_w"]
    rows_pad = 128 * n_strips
    sdf_f = cfg["sdf_f"]
    eik_f = cfg["eik_f"]
    pair_f = cfg["pair_f"]

    nc = bacc.Bacc("TRN2", target_bir_lowering=False)

    # ---- inputs ----
    d_aA = nc.dram_tensor("a_a", [11, rows_pad], bf16, kind="ExternalInput")
    d_bA = nc.dram_tensor("b_a", [11, slice_w], bf16, kind="ExternalInput")
    d_aB = nc.dram_tensor("a_b", [11, rows_pad], bf16, kind="ExternalInput")
    d_bB = nc.dram_tensor("b_b", [11, slice_w], bf16, kind="ExternalInput")
    d_sdf_pred = nc.dram_tensor("sdf_pred", [128, sdf_f], f32, kind="ExternalInput")
    d_sdf_gt = nc.dram_tensor("sdf_gt", [128, sdf_f], f32, kind="ExternalInput")
    d_eik_pred = nc.dram_tensor("eik_pred", [128 * eik_f + 1], f32, kind="ExternalInput")
    d_eik_gt = nc.dram_tensor("eik_gt", [128, eik_f], f32, kind="ExternalInput")
    d_edge = nc.dram_tensor("edge_in", [18, 128, pair_f], f32, kind="ExternalInput")

    # ---- outputs ----
    d_chamA = nc.dram_tensor("cham_a", [128, n_strips * nsub], f32, kind="ExternalOutput")
    d_chamB = nc.dram_tensor("cham_b", [128, n_strips * nsub], f32, kind="ExternalOutput")
    # part_out cols: 0 sdf_absdiff, 1 sdf_4e_absdiff, 2 eik_num, 3 eik_cnt, 4 edge_relu
    d_part = nc.dram_tensor("part_out", [128, 8], f32, kind="ExternalOutput")

    with tile.TileContext(nc) as tc, ExitStack() as ctx:
        singles = ctx.enter_context(tc.tile_pool(name="singles", bufs=1))
        cpool = ctx.enter_context(tc.tile_pool(name="cpool", bufs=1))
        epool = ctx.enter_context(tc.tile_pool(name="epool", bufs=1))
        spool = ctx.enter_context(tc.tile_pool(name="spool", bufs=1))
        psum = ctx.enter_context(tc.tile_pool(name="psum", bufs=4, space="PSUM"))

        chamA_o = singles.tile([128, n_strips * nsub], f32)
        chamB_o = singles.tile([128, n_strips * nsub], f32)
        part_o = singles.tile([128, 8], f32)

        # ---- input DMAs, spread across engine queues for parallel issue ----
        ev = epool.tile([128, 18, pair_f], f32)
        for h in range(2):   # two halves (face A planes / face B planes)
            src = bass.AP(tensor=d_edge[:, :, :].tensor, offset=h * 9 * 128 * pair_f,
                          ap=[[pair_f, 128], [128 * pair_f, 9], [1, pair_f]])
            (nc.gpsimd if h == 0 else nc.scalar).dma_start(
                out=ev[:, h * 9:(h + 1) * 9, :], in_=src)

        aA_t = cpool.tile([11, rows_pad], bf16)
        bA_t = cpool.tile([11, slice_w], bf16)
        aB_t = cpool.tile([11, rows_pad], bf16)
        bB_t = cpool.tile([11, slice_w], bf16)
        nc.sync.dma_start(out=aA_t, in_=d_aA[:, :])
        nc.sync.dma_start(out=bA_t, in_=d_bA[:, :])
        nc.sync.dma_start(out=aB_t, in_=d_aB[:, :])
        nc.sync.dma_start(out=bB_t, in_=d_bB[:, :])

        sdf_pr = spool.tile([128, sdf_f], f32)
        sdf_g = spool.tile([128, sdf_f], f32)
        nc.scalar.dma_start(out=sdf_pr, in_=d_sdf_pred[:, :])
        nc.scalar.dma_start(out=sdf_g, in_=d_sdf_gt[:, :])

        ep0 = spool.tile([128, eik_f], f32)
        ep1 = spool.tile([128, eik_f], f32)
        eg = spool.tile([128, eik_f], f32)
        base = d_eik_pred[:]
        src0 = bass.AP(tensor=base.tensor, offset=0, ap=[[eik_f, 128], [1, eik_f]])
        src1 = bass.AP(tensor=base.tensor, offset=1, ap=[[eik_f, 128], [1, eik_f]])
        nc.sync.dma_start(out=ep0[:, :], in_=src0)
        nc.sync.dma_start(out=ep1[:, :], in_=src1)
        nc.sync.dma_start(out=eg, in_=d_eik_gt[:, :])

        nc.vector.memset(part_o, 0.0)

        # ============ sdf + eikonal elementwise (GpSimd, before edge) ========
        # sdf clips
        prc = spool.tile([128, sdf_f], f32)
        gc = spool.tile([128, sdf_f], f32)
        nc.gpsimd.tensor_scalar(out=prc, in0=sdf_pr, scalar1=TRUNC, scalar2=-TRUNC,
                                op0=OP.min, op1=OP.max)
        nc.gpsimd.tensor_scalar(out=gc, in0=sdf_g, scalar1=TRUNC, scalar2=-TRUNC,
                                op0=OP.min, op1=OP.max)
        diff = spool.tile([128, sdf_f], f32)
        nc.gpsimd.tensor_tensor(out=diff, in0=prc, in1=gc, op=OP.subtract)
        # eikonal elementwise
        dx = spool.tile([128, eik_f], f32)
        nc.gpsimd.tensor_tensor(out=dx, in0=ep1[:, :], in1=ep0[:, :], op=OP.subtract)
        abseg = spool.tile([128, eik_f], f32)
        nc.scalar.activation(out=abseg, in_=eg, func=AF.Abs)
        mask = spool.tile([128, eik_f], f32)
        nc.gpsimd.tensor_scalar(out=mask, in0=abseg, scalar1=TRUNC, scalar2=None,
                                op0=OP.is_lt)

        # ================= chamfer (banded, both directions) =================
        for s in range(n_strips):
            for (a_t, b_t, out_t) in ((aA_t, bA_t, chamA_o), (aB_t, bB_t, chamB_o)):
                ps = psum.tile([128, win], f32, tag="ps")
                for m in range(win // 512):
                    nc.tensor.matmul(ps[:, m * 512:(m + 1) * 512],
                                     a_t[:, s * 128:(s + 1) * 128],
                                     b_t[:, s * 128 + m * 512: s * 128 + (m + 1) * 512],
                                     start=True, stop=True)
                ps_ap = ps[:, :]
                ps3d = bass.AP(tensor=ps_ap.tensor, offset=ps_ap.offset,
                               ap=[ps_ap.ap[0], [sub, nsub], [1, sub]])
                nc.vector.tensor_reduce(out=out_t[:, s * nsub:(s + 1) * nsub],
                                        in_=ps3d, axis=AX.X, op=OP.max)

        # ================= edge loss (float part, GpSimd packed) =============
        _emit_edge(nc, epool, part_o, ev, pair_f, f32, AX, OP, AF)

        # ================= sdf + eikonal finishers ===========================
        absdiff = spool.tile([128, sdf_f], f32)
        nc.scalar.activation(out=absdiff, in_=diff, func=AF.Abs)
        nc.vector.tensor_reduce(out=part_o[:, 0:1], in_=absdiff, axis=AX.X, op=OP.add)
        absg = spool.tile([128, sdf_f], f32)
        nc.scalar.activation(out=absg, in_=gc, func=AF.Abs)
        e = spool.tile([128, sdf_f], f32)
        nc.scalar.activation(out=e, in_=absg, func=AF.Exp, scale=-1.0 / SIGMA)
        dead = spool.tile([128, sdf_f], f32)
        nc.vector.scalar_tensor_tensor(out=dead, in0=e, scalar=SURF_W - 1.0,
                                       in1=absdiff, op0=OP.mult, op1=OP.mult,
                                       accum_out=part_o[:, 1:2])

        absdx = spool.tile([128, eik_f], f32)
        nc.scalar.activation(out=absdx, in_=dx, func=AF.Abs)
        t = spool.tile([128, eik_f], f32)
        nc.vector.tensor_scalar(out=t, in0=absdx, scalar1=-1.0, scalar2=None,
                                op0=OP.add)
        t2 = spool.tile([128, eik_f], f32)
        nc.vector.tensor_tensor(out=t2, in0=t, in1=t, op=OP.mult)
        mt2 = spool.tile([128, eik_f], f32)
        nc.vector.tensor_tensor(out=mt2, in0=t2, in1=mask, op=OP.mult)
        nc.vector.tensor_reduce(out=part_o[:, 2:3], in_=mt2, axis=AX.X, op=OP.add)
        nc.vector.tensor_reduce(out=part_o[:, 3:4], in_=mask, axis=AX.X, op=OP.add)

        nc.sync.dma_start(out=d_chamA[:, :], in_=chamA_o[:, :])
        nc.sync.dma_start(out=d_chamB[:, :], in_=chamB_o[:, :])
        nc.sync.dma_start(out=d_part[:, :], in_=part_o[:, :])

    nc.compile()
    return nc


def _emit_edge(nc, epool, part_o, ev, pair_f, f32, AX, OP, AF):
    """Edge-pair dihedral loss.  GpSimd does the [128,3,120]-packed vector
    algebra; DVE finishes the small [128,120] tail; ScalarE sqrt/relu."""
    P = pair_f

    def gp_tt(out, in0, in1, op):
        nc.gpsimd.tensor_tensor(out=out, in0=in0, in1=in1, op=op)

    # edge vectors with rotated duplicates: layout [128, 5, P] = [x,y,z,x,y]
    evecs = {}
    for name, vbase, fbase in (("e1A", 3, 0), ("e2A", 6, 0),
                               ("e1B", 3, 9), ("e2B", 6, 9)):
        buf = epool.tile([128, 5, P], f32, name=f"ev_{name}")
        gp_tt(buf[:, 0:3, :], ev[:, fbase + vbase:fbase + vbase + 3, :],
              ev[:, fbase:fbase + 3, :], OP.subtract)
        nc.gpsimd.tensor_copy(out=buf[:, 3:5, :], in_=buf[:, 0:2, :])
        evecs[name] = buf

    def cross(e1, e2, name):
        t1 = epool.tile([128, 3, P], f32, name=f"cx1_{name}")
        t2 = epool.tile([128, 3, P], f32, name=f"cx2_{name}")
        out = epool.tile([128, 3, P], f32, name=f"n_{name}")
        gp_tt(t1, e1[:, 1:4, :], e2[:, 2:5, :], OP.mult)
        gp_tt(t2, e1[:, 2:5, :], e2[:, 1:4, :], OP.mult)
        gp_tt(out, t1, t2, OP.subtract)
        return out

    na = cross(evecs["e1A"], evecs["e2A"], "A")
    nb = cross(evecs["e1B"], evecs["e2B"], "B")

    def dot3(a, b, name):
        m = epool.tile([128, 3, P], f32, name=f"dm_{name}")
        gp_tt(m, a, b, OP.mult)
        s01 = epool.tile([128, P], f32, name=f"ds_{name}")
        nc.vector.tensor_tensor(out=s01, in0=m[:, 0, :], in1=m[:, 1, :], op=OP.add)
        s = epool.tile([128, P], f32, name=f"dt_{name}")
        nc.vector.tensor_tensor(out=s, in0=s01, in1=m[:, 2, :], op=OP.add)
        return s

    dot = dot3(na, nb, "AB")
    na2 = dot3(na, na, "AA")
    nb2 = dot3(nb, nb, "BB")
    prod2 = epool.tile([128, P], f32)
    nc.vector.tensor_tensor(out=prod2, in0=na2, in1=nb2, op=OP.mult)
    sa = epool.tile([128, P], f32)
    nc.scalar.activation(out=sa, in_=prod2, func=AF.Sqrt)
    sac = epool.tile([128, P], f32)
    nc.vector.tensor_scalar(out=sac, in0=sa, scalar1=1e-24, scalar2=None, op0=OP.max)
    rs = epool.tile([128, P], f32)
    nc.vector.reciprocal(out=rs, in_=sac)
    cos = epool.tile([128, P], f32)
    nc.vector.tensor_tensor(out=cos, in0=dot, in1=rs, op=OP.mult)
    relu = epool.tile([128, P], f32)
    nbias = epool.tile([128, 1], f32)
    nc.vector.memset(nbias, -DIH_THR)
    nc.scalar.activation(out=relu, in_=cos, func=AF.Relu, bias=nbias[:, 0:1],
                         accum_out=part_o[:, 4:5])


def get_program(cfg_key="full"):
    if cfg_key not in _PROG_CACHE:
        _PROG_CACHE[cfg_key] = build_program(FULL_CFG)
    return _PROG_CACHE[cfg_key]


# ================================================================== host side
def _hi_lo(x):
    h = x.astype(BF16)
    l = (x - h.astype(np.float64)).astype(BF16)
    return h, l


def _build_lhs(a):
    """a: [n,3] fp64 -> [11,n] bf16 rows [ah3, ah3, al3, 1, 1]."""
    ah, al = _hi_lo(a)
    ones = np.ones((1, a.shape[0]), BF16)
    return np.ascontiguousarray(
        np.concatenate([ah.T, ah.T, al.T, ones, ones], 0))


def _build_rhs(b):
    """b: [m,3] fp64 -> [11,m] bf16 rows [2bh3, 2bl3, 2bh3, -sh, -sl]."""
    bh = b.astype(BF16)
    bl2 = (2.0 * (b - bh.astype(np.float64))).astype(BF16)
    bh2 = (2.0 * bh.astype(np.float64)).astype(BF16)
    s = (b * b).sum(-1)
    sh = s.astype(BF16)
    sl = (s - sh.astype(np.float64)).astype(BF16)
    neg_sh = (-sh.astype(np.float64)).astype(BF16)
    neg_sl = (-sl.astype(np.float64)).astype(BF16)
    return np.ascontiguousarray(
        np.concatenate([bh2.T, bl2.T, bh2.T, neg_sh[None], neg_sl[None]], 0))


def _host_prep(inputs, cfg):
    np_f32 = np.float32
    npts = cfg["npts"]
    shard = cfg["shard"]
    n_strips = cfg["n_strips"]
    rows_pad = 128 * n_strips
    slice_w = cfg["slice_w"]
    padl = cfg["padl"]
    ext_len = cfg["ext_len"]

    pred_pts = np.asarray(inputs["pred_points"][0], dtype=np.float64)
    gt_pts = np.asarray(inputs["gt_points"][0], dtype=np.float64)

    pperm = np.argsort(pred_pts[:, 0], kind="stable")
    gperm = np.argsort(gt_pts[:, 0], kind="stable")
    ps = pred_pts[pperm]
    gs = gt_pts[gperm]

    def make_ext(sorted_pts):
        ext = np.empty((ext_len, 3))
        ext[:padl] = [-1e9, 0.0, 0.0]
        ext[padl:padl + npts] = sorted_pts
        ext[padl + npts:] = [1e9, 0.0, 0.0]
        return ext

    g_ext = make_ext(gs)
    p_ext = make_ext(ps)

    def pad_rows(x, n):
        out = np.zeros((n, 3))
        out[:x.shape[0]] = x
        return out

    rhs_gt = _build_rhs(g_ext)     # [11, ext_len]
    rhs_pr = _build_rhs(p_ext)

    # --- sdf / eikonal shards (unsorted originals) ---
    pred_sdf = inputs["pred_sdf"].reshape(-1).astype(np_f32)
    gt_sdf = inputs["gt_sdf"].reshape(-1).astype(np_f32)
    n_tot = pred_sdf.shape[0]
    sdf_shard, sdf_f, eik_f = cfg["sdf_shard"], cfg["sdf_f"], cfg["eik_f"]
    n_batch = inputs["pred_sdf"].shape[1]

    # --- edge pairing on host (int32 faces only) ---
    verts = np.asarray(inputs["extracted_vertices"], dtype=np_f32)
    faces = np.asarray(inputs["extracted_faces"], dtype=np.int64)
    V = verts.shape[0]
    Fn = faces.shape[0]
    a = faces
    b = np.roll(faces, -1, axis=1)
    lo = np.minimum(a, b)
    hi = np.maximum(a, b)
    key = (lo * V + hi).reshape(-1)
    fid = np.repeat(np.arange(Fn, dtype=np.int64), 3)
    order = np.argsort(key, kind="stable")
    k = key[order]
    f = fid[order]
    same_next = k[:-1] == k[1:]
    prev = np.concatenate([[False], same_next[:-1]])
    nxt = np.concatenate([same_next[1:], [False]])
    is_pair = same_next & ~prev & ~nxt
    pos = np.nonzero(is_pair)[0]
    fa = f[pos]
    fb = f[pos + 1]
    npairs = int(pos.shape[0])
    is_start = np.concatenate([[True], k[1:] != k[:-1]])
    starts = np.nonzero(is_start)[0]
    run_len = np.diff(np.concatenate([starts, [k.shape[0]]]))
    total_unique = int(starts.shape[0])
    bad = int((run_len != 2).sum())
    wt = (bad / total_unique) if total_unique > 0 else 0.0

    pair_cap = cfg["pair_cap"]
    n_dev_pairs = min(npairs, pair_cap)
    planes = np.zeros((18, pair_cap), np_f32)
    if n_dev_pairs > 0:
        va = verts[faces[fa[:n_dev_pairs]]]
        vb = verts[faces[fb[:n_dev_pairs]]]
        planes[0:9, :n_dev_pairs] = va.reshape(n_dev_pairs, 9).T
        planes[9:18, :n_dev_pairs] = vb.reshape(n_dev_pairs, 9).T
    edge_extra = 0.0
    if npairs > pair_cap:
        va = verts[faces[fa[pair_cap:]]]
        vb = verts[faces[fb[pair_cap:]]]
        na = np.cross(va[:, 1] - va[:, 0], va[:, 2] - va[:, 0])
        nb = np.cross(vb[:, 1] - vb[:, 0], vb[:, 2] - vb[:, 0])
        na /= np.maximum(np.linalg.norm(na, axis=-1, keepdims=True), 1e-12)
        nb /= np.maximum(np.linalg.norm(nb, axis=-1, keepdims=True), 1e-12)
        cosv = (na * nb).sum(-1)
        edge_extra = float(np.maximum(cosv - DIH_THR, 0.0).sum())

    pair_f = cfg["pair_f"]
    planes8 = planes.reshape(18, N_CORES, 128 * pair_f).transpose(1, 0, 2) \
                    .reshape(N_CORES, 18, 128, pair_f)

    in_maps = []
    for c in range(N_CORES):
        lhsA = _build_lhs(pad_rows(ps[c * shard:(c + 1) * shard], rows_pad))
        lhsB = _build_lhs(pad_rows(gs[c * shard:(c + 1) * shard], rows_pad))
        bA = np.ascontiguousarray(rhs_gt[:, c * shard: c * shard + slice_w])
        bB = np.ascontiguousarray(rhs_pr[:, c * shard: c * shard + slice_w])

        sp = np.zeros(128 * sdf_f, np_f32)
        sg = np.zeros(128 * sdf_f, np_f32)
        sl = pred_sdf[c * sdf_shard:(c + 1) * sdf_shard]
        sp[:sl.shape[0]] = sl
        sg[:sl.shape[0]] = gt_sdf[c * sdf_shard:(c + 1) * sdf_shard]

        ep = np.zeros(128 * eik_f + 1, np_f32)
        src = pred_sdf[c * sdf_shard: c * sdf_shard + 128 * eik_f + 1]
        ep[:src.shape[0]] = src
        eg = np.full(128 * eik_f, 1e9, np_f32)
        gsrc = gt_sdf[c * sdf_shard: c * sdf_shard + 128 * eik_f]
        eg[:gsrc.shape[0]] = gsrc
        locs = np.arange(128 * eik_f)
        glob = locs + c * sdf_shard
        bad_m = (locs >= sdf_shard) | ((glob % n_batch) == n_batch - 1) | \
                (glob >= n_tot - 1)
        eg[bad_m] = 1e9

        in_maps.append({
            "a_a": lhsA,
            "b_a": bA,
            "a_b": lhsB,
            "b_b": bB,
            "sdf_pred": sp.reshape(128, sdf_f),
            "sdf_gt": sg.reshape(128, sdf_f),
            "eik_pred": ep,
            "eik_gt": eg.reshape(128, eik_f),
            "edge_in": np.ascontiguousarray(planes8[c]),
        })

    meta = dict(npairs=npairs, wt=wt, edge_extra=edge_extra,
                pperm=pperm, gperm=gperm, ps=ps, gs=gs,
                p_ext=p_ext, g_ext=g_ext)
    return in_maps, meta


def _refine_side(cfg, results, key, qs, ext, t_sorted):
    """Host top-2 subtile refinement + optimality proof + bounded fallback.

    qs: sorted query points [N,3] fp64; ext: target ext array [ext_len,3];
    t_sorted: sorted target points [N,3].  Returns (d2_min[N],
    argmin_rank[N], n_fallback) in SORTED-query order.
    """
    npts = cfg["npts"]
    shard = cfg["shard"]
    n_strips = cfg["n_strips"]
    sub = cfg["sub"]
    nsub = cfg["win"] // sub
    padl = cfg["padl"]
    ext_len = cfg["ext_len"]
    W = cfg["win"]

    M = np.empty((npts, nsub), np.float32)
    wstart = np.empty(npts, np.int64)
    lr = np.arange(shard)
    for c in range(N_CORES):
        cham = np.asarray(results[c][key])            # [128, n_strips*nsub]
        # [p, s*nsub+t] -> local row s*128+p
        loc = cham.reshape(128, n_strips, nsub).transpose(1, 0, 2).reshape(-1, nsub)
        M[c * shard:(c + 1) * shard] = loc[:shard]
        wstart[c * shard:(c + 1) * shard] = c * shard + 128 * (lr // 128)

    top2 = np.argpartition(-M, 1, axis=1)[:, :2]
    cand = wstart[:, None, None] + top2[:, :, None] * sub + np.arange(sub)[None, None, :]
    cand = cand.reshape(npts, 2 * sub)
    tc = ext[cand]
    d2 = ((qs[:, None, :] - tc) ** 2).sum(-1)
    kk = np.argmin(d2, axis=1)
    dmin = d2[np.arange(npts), kk]
    ecol = cand[np.arange(npts), kk]

    tx = ext[:, 0]
    safeL = np.where(wstart == 0, np.inf, qs[:, 0] - tx[np.maximum(wstart - 1, 0)])
    wend = wstart + W
    safeR = np.where(wend >= ext_len, np.inf, tx[np.minimum(wend, ext_len - 1)] - qs[:, 0])
    safe = np.maximum(np.minimum(safeL, safeR), 0.0)
    fb = np.nonzero(dmin > safe * safe)[0]
    if fb.size:
        # exact bounded re-scan: the true NN must satisfy |x_q - x_t| <= sqrt(d_band)
        d = np.sqrt(dmin[fb])
        txs = t_sorted[:, 0]
        lo = np.searchsorted(txs, qs[fb, 0] - d, side="left")
        hi = np.searchsorted(txs, qs[fb, 0] + d, side="right")
        maxw = max(1, int((hi - lo).max()))
        cols = lo[:, None] + np.arange(maxw)[None, :]
        valid = cols < hi[:, None]
        cols = np.minimum(cols, npts - 1)
        tcf = t_sorted[cols]
        dd = ((qs[fb, None, :] - tcf) ** 2).sum(-1)
        dd[~valid] = np.inf
        kf = np.argmin(dd, axis=1)
        dmin[fb] = dd[np.arange(fb.size), kf]
        ecol[fb] = cols[np.arange(fb.size), kf] + padl
    rank = ecol - padl
    return dmin, rank, int(fb.size)


def _host_post(inputs, cfg, results, meta):
    npts = cfg["npts"]
    pperm, gperm = meta["pperm"], meta["gperm"]
    ps, gs = meta["ps"], meta["gs"]

    dA, rankA, _ = _refine_side(cfg, results, "cham_a", ps, meta["g_ext"], gs)
    dB, _, _ = _refine_side(cfg, results, "cham_b", gs, meta["p_ext"], ps)
    ch = dA.mean() + dB.mean()

    # normal consistency: map sorted-query rows back to original indices
    idxA = np.empty(npts, np.int64)
    idxA[pperm] = gperm[np.clip(rankA, 0, npts - 1)]
    pn = inputs["pred_normals"][0].astype(np.float64)
    gn = inputs["gt_normals"][0].astype(np.float64)
    matched = gn[idxA]
    eps = 1e-8
    num = (pn * matched).sum(-1)
    den = np.maximum(np.linalg.norm(pn, axis=-1), eps) * \
        np.maximum(np.linalg.norm(matched, axis=-1), eps)
    nrm = float(np.mean(1.0 - np.abs(num / den)))

    parts = np.stack([np.asarray(results[c]["part_out"]) for c in range(N_CORES)])
    psum = parts.astype(np.float64).sum(axis=(0, 1))
    sdf = (psum[0] + psum[1]) / float(cfg["sdf_n"])
    eik = (psum[2] / psum[3]) if psum[3] > 0 else 0.0

    npairs = meta["npairs"]
    edge = ((psum[4] + meta["edge_extra"]) / npairs) if npairs > 0 else 0.0

    total = (SDF_W * sdf + EIK_W * eik + CH_W * ch + NORM_W * nrm +
             EDGE_W * edge + WT_W * meta["wt"])
    return np.asarray(np.float32(total))


def kernel(**inputs):
    from concourse.bass_utils import run_bass_kernel_spmd
    cfg = FULL_CFG
    nc = get_program()
    in_maps, meta = _host_prep(inputs, cfg)
    res = run_bass_kernel_spmd(nc, in_maps, core_ids=list(range(N_CORES)))
    return _host_post(inputs, cfg, res.results, meta)
